# revision 22
# baseline (speedup 1.0000x reference)
"""Multi-head attention forward on 8 Trainium2 NeuronCores (Bass/Tile).

Problem: x[2,2048,1024] -> qkv (w_qkv [3072,1024]) -> 16-head softmax attention
-> proj (w_proj [1024,1024] + b_proj) -> out[2,2048,1024], fp32.

Sharding: head-parallel. Core c computes heads (2c, 2c+1) for BOTH batches:
its slice of the QKV projection, its heads' attention, then two 8-core
AllToAlls (one per batch; the batch-0 one overlaps batch-1 attention)
redistribute the attention output from head-split to token-split, after which
each core does the full-depth output projection (+bias) for its 2x256-token
striped slice. Host assembles the 16 stripes.

Layout: everything flows transposed (d-major / "T" form):
  - host feeds xT [1024, 4096] and per-core w_qkvT [1024, 384] (softmax scale
    folded into the q columns), w_projT [1024, 1024], all bf16 (fp32 matmuls
    stream at half rate on the PE).
  - QKV phase emits QT/KT/VT [128 (2 heads x 64d), 4096 tokens] e-major; VT is
    then PE-transposed per 128-token chunk into V [token-major, 2 x (64 + ones
    column)] for the PV matmul (col 64 of each 65-block -> softmax denom).
  - scores are computed transposed (scoresT[k, q] = sum_d KT[d,k] QT[d,q]),
    two heads row-packed in the PE array (K=64 each, rows 0-63 / 64-127).
    Softmax runs WITHOUT max subtraction (logits ~ N(0,1); exp is safe in
    fp32), and the PV matmul's ones-column yields the denominators free.
"""

import numpy as np

import concourse.bass as bass
import concourse.tile as tile
from concourse import bacc, mybir
from concourse.bass_utils import run_bass_kernel_spmd

F32 = mybir.dt.float32
BF16 = mybir.dt.bfloat16
ts = bass.ts

B = 2
S = 2048
D = 1024
H = 16
HD = 64
SCALE = HD ** -0.5
T = B * S  # 4096 tokens
N_CORES = 8

TT = 512  # phase-A token tile (matmul N)
N_TT = T // TT  # 8
KC = 128  # attention key-chunk (contraction tile for PV)
N_KC = S // KC  # 16 per batch
QT = 1024  # attention query tile (exp instruction width)
N_QT = S // QT  # 2 per batch
STRIPE = 256  # per-core tokens per batch (striped output slices)


def build_attention_body(ctx, tc, xT, w_qkv, w_pj, b_pj, ident, out, taps=None):
    nc = tc.nc
    EXP = mybir.ActivationFunctionType.Exp

    singles = ctx.enter_context(tc.tile_pool(name="singles", bufs=1))
    dram = ctx.enter_context(tc.tile_pool(name="dram", bufs=1, space="DRAM"))

    # persistent SBUF
    # qk_sb[0] = QT-pair [128 (h0 d0-63 | h1 d0-63), 4096], qk_sb[1] = KT-pair
    qk_sb = [singles.tile([128, T], BF16, name=f"qk_sb{i}") for i in range(2)]
    # V: per 128-token chunk kcg and head hh, [128 k, 64] at cols 64*(2*kcg+hh)
    v_sb = singles.tile([128, 2 * B * N_KC * 64], BF16, name="v_sb")
    # attention output (normalized), d-major: head A rows 0-63, head B 64-127
    outT = [singles.tile([128, S], BF16, name=f"outT_{b_}") for b_ in range(B)]
    wqkv_sb = [singles.tile([128, 384], BF16, name=f"wqkv_sb{dc}") for dc in range(8)]
    wpj_sb = [singles.tile([128, D], BF16, name=f"wpj_sb{dc}") for dc in range(8)]
    bias_sb = singles.tile([1, D], BF16, name="bias_sb")
    ones1 = singles.tile([1, 128], BF16, name="ones1")
    ones_col = singles.tile([128, 1], BF16, name="ones_col")
    id_sb = singles.tile([128, 128], BF16, name="id_sb")

    # DRAM bounce buffers for the two AllToAlls (batch 0, batch 1)
    a2a_in = [
        dram.tile([N_CORES * 128, STRIPE], BF16, name=f"a2a_in{b_}") for b_ in range(B)
    ]
    a2a_out = [
        dram.tile([N_CORES * 128, STRIPE], BF16, name=f"a2a_out{b_}") for b_ in range(B)
    ]
    rdram = ctx.enter_context(tc.tile_pool(name="rdram", bufs=4, space="DRAM"))

    # ---- load weights / constants
    for dc in range(8):
        nc.sync.dma_start(wqkv_sb[dc][:], w_qkv[ts(dc, 128), :])
        nc.sync.dma_start(wpj_sb[dc][:], w_pj[ts(dc, 128), :])
    nc.sync.dma_start(bias_sb[:], b_pj[:])
    nc.sync.dma_start(id_sb[:], ident[:])
    nc.vector.memset(ones1[:], 1.0)
    nc.vector.memset(ones_col[:], 1.0)

    # ---- Phase A: QKV projections, all e-major; V transposed on the PE
    xpool = ctx.enter_context(tc.tile_pool(name="xt", bufs=16))
    vt_pool = ctx.enter_context(tc.tile_pool(name="vt", bufs=2))
    with tc.tile_pool(name="psA", bufs=3, space="PSUM") as psA_pool:
        with tc.tile_pool(name="psT", bufs=2, space="PSUM") as psT_pool:
            for tt in range(N_TT):
                xts = []
                for dc in range(8):
                    xt_t = xpool.tile([128, TT], BF16, tag="xt", name=f"xt_{tt}_{dc}")
                    nc.sync.dma_start(xt_t[:], xT[ts(dc, 128), ts(tt, TT)])
                    xts.append(xt_t)
                # out[e, t] = sum_d w_qkvT[d, e] * xT[d, t], e-tiles: Q | K | V
                vt_t = vt_pool.tile([128, TT], BF16, tag="vt", name=f"vt_{tt}")
                for et in range(3):
                    ps = psA_pool.tile([128, TT], F32, tag="qk", name=f"ps_{tt}_{et}")
                    for dc in range(8):
                        nc.tensor.matmul(
                            ps[:],
                            lhsT=wqkv_sb[dc][:, ts(et, 128)],
                            rhs=xts[dc][:],
                            start=(dc == 0),
                            stop=(dc == 7),
                        )
                    if et < 2:
                        nc.scalar.copy(qk_sb[et][:, ts(tt, TT)], ps[:])
                    else:
                        nc.scalar.copy(vt_t[:], ps[:])
                # transpose VT 128-token chunks into v_sb
                for st in range(TT // 128):
                    kcg = tt * (TT // 128) + st  # global 128-token chunk 0..31
                    tp = psT_pool.tile([128, 128], BF16, tag="tp", name=f"tp_{kcg}")
                    nc.tensor.transpose(tp[:], vt_t[:, ts(st, 128)], id_sb[:])
                    nc.vector.tensor_copy(
                        v_sb[:, 128 * kcg : 128 * kcg + 128], tp[:]
                    )

    if taps is not None:
        nc.sync.dma_start(taps["qk0"][:], qk_sb[0][:])
        nc.sync.dma_start(taps["qk1"][:], qk_sb[1][:])
        nc.sync.dma_start(taps["v"][:], v_sb[:])

    # ---- Phase B: attention per (batch, query-tile); batch-b A2A fires as
    # soon as batch b's divisions are done (b=0's overlaps b=1's attention)
    probs = ctx.enter_context(tc.tile_pool(name="probs", bufs=2))
    misc = ctx.enter_context(tc.tile_pool(name="misc", bufs=2))
    with tc.tile_pool(name="psS", bufs=2, space="PSUM") as psS_pool:
        with tc.tile_pool(name="psP", bufs=2, space="PSUM") as psP_pool:
            with tc.tile_pool(name="psD", bufs=2, space="PSUM") as psD_pool:
                for b_ in range(B):
                    for qt in range(N_QT):
                        q0 = S * b_ + QT * qt
                        # col-packed accumulators: head A rows 0-63, head B
                        # rows 64-127 of the SAME bank. Only head A's kc==0
                        # matmul uses start=True (it clears the whole bank's
                        # has_written bits); head B always start=False and
                        # relies on overwrite-where-unset for its first write.
                        pvs = [
                            psP_pool.tile(
                                [128, 512], F32, tag="pv", name=f"pv_{b_}_{qt}_{qs}"
                            )
                            for qs in range(QT // 512)
                        ]
                        dens = [
                            psD_pool.tile(
                                [128, 512], F32, tag="den", name=f"den_{b_}_{qt}_{qs}"
                            )
                            for qs in range(QT // 512)
                        ]
                        # zero head B's regions: its first matmul never uses
                        # start=True (that would clear head A's has_written
                        # bits bank-wide); overwrite-where-unset on HW and
                        # plain accumulate in CoreSim both land on the right
                        # values when the region starts at zero
                        for qs in range(QT // 512):
                            nc.vector.memset(pvs[qs][:], 0.0)
                            # 1.0 (not 0.0) so the reciprocal over the unused
                            # rows 1-63 stays finite for the simulator; head
                            # B's first den matmul overwrites row 64 on HW
                            # (has_written unset) and accumulates onto 1.0 in
                            # CoreSim (negligible: denominators are ~2e3)
                            nc.vector.memset(dens[qs][:], 1.0)

                        def emit_pv(kc, prs, b_=b_, pvs=pvs, dens=dens):
                            first = kc == 0
                            last = kc == N_KC - 1
                            for qs in range(QT // 512):
                                for hh in range(2):
                                    blk = 64 * (2 * (N_KC * b_ + kc) + hh)
                                    nc.tensor.matmul(
                                        pvs[qs][64 * hh : 64 * hh + 64, :],
                                        lhsT=v_sb[:, blk : blk + 64],
                                        rhs=prs[hh][:, ts(qs, 512)],
                                        start=(first and hh == 0),
                                        stop=(last and hh == 1),
                                        skip_group_check=True,
                                    )
                            for qs in range(QT // 512):
                                for hh in range(2):
                                    nc.tensor.matmul(
                                        dens[qs][64 * hh : 64 * hh + 1, :],
                                        lhsT=ones_col[:],
                                        rhs=prs[hh][:, ts(qs, 512)],
                                        start=(first and hh == 0),
                                        stop=(last and hh == 1),
                                        skip_group_check=True,
                                    )

                        prev = None
                        for kc in range(N_KC):
                            k0 = S * b_ + KC * kc
                            # both heads' scores in ONE 4-bank psum tile:
                            # head hh occupies cols [QT*hh, QT*hh+QT)
                            sc = psS_pool.tile(
                                [128, 2 * QT], F32, tag="sc", bufs=1,
                                name=f"sc_{b_}_{qt}_{kc}",
                            )
                            # interleave heads so row-packed pairs co-issue
                            for qs in range(QT // 512):
                                for hh in range(2):
                                    p0 = 64 * hh
                                    nc.tensor.matmul(
                                        sc[:, QT * hh + 512 * qs : QT * hh + 512 * qs + 512],
                                        lhsT=qk_sb[1][p0 : p0 + 64, k0 : k0 + KC],
                                        rhs=qk_sb[0][
                                            p0 : p0 + 64,
                                            q0 + 512 * qs : q0 + 512 * qs + 512,
                                        ],
                                        start=True,
                                        stop=True,
                                    )
                            # one exp covers both heads (2048 wide)
                            pr = probs.tile(
                                [128, 2 * QT], BF16, tag="pr", bufs=2,
                                name=f"pr_{b_}_{qt}_{kc}",
                            )
                            nc.scalar.activation(pr[:], sc[:], EXP)
                            prs = [pr[:, 0:QT], pr[:, QT : 2 * QT]]
                            if prev is not None:
                                emit_pv(kc - 1, prev)
                            prev = prs
                        emit_pv(N_KC - 1, prev)

                        # normalize: rows 0-63 (head A) / 64-127 (head B) by
                        # the denominators in dens rows 0 / 64
                        for qs in range(QT // 512):
                            den_t = dens[qs]
                            pv_t = pvs[qs]
                            # the approx reciprocal must start at partition
                            # 0 (custom-DVE ops mis-execute at base_partition
                            # != 0 on HW); rows 1-63 hold the memset 1.0
                            rc = misc.tile(
                                [128, 512], F32, tag="rc", name=f"rc_{b_}_{qt}_{qs}"
                            )
                            nc.vector.reciprocal_approx_fast(
                                rc[0:65, :], den_t[0:65, :]
                            )
                            bc = misc.tile(
                                [128, 512], F32, tag="bc", name=f"bc_{b_}_{qt}_{qs}"
                            )
                            for hh in range(2):
                                rd = rdram.tile(
                                    [1, 512], F32, tag="rd",
                                    name=f"rd_{b_}_{qt}_{qs}_{hh}",
                                )
                                nc.sync.dma_start(rd[:], rc[64 * hh : 64 * hh + 1, :])
                                nc.sync.dma_start(
                                    bc[64 * hh : 64 * hh + 64, :],
                                    rd[:].to_broadcast((64, 512)),
                                )
                            qq = QT * qt + 512 * qs
                            for hh in range(2):
                                sl = slice(64 * hh, 64 * hh + 64)
                                nc.vector.tensor_mul(
                                    outT[b_][sl, qq : qq + 512],
                                    pv_t[sl, :],
                                    bc[sl, :],
                                )

                    # batch b_ fully normalized -> ship its AllToAll now
                    for j in range(N_CORES):
                        nc.sync.dma_start(
                            a2a_in[b_][128 * j : 128 * j + 128, :],
                            outT[b_][:, ts(j, STRIPE)],
                        )
                    nc.gpsimd.collective_compute(
                        "AllToAll",
                        mybir.AluOpType.bypass,
                        replica_groups=[list(range(N_CORES))],
                        ins=[a2a_in[b_].opt()],
                        outs=[a2a_out[b_].opt()],
                    )

    if taps is not None:
        for b_ in range(B):
            nc.sync.dma_start(taps[f"outT{b_}"][:], outT[b_][:])
        nc.sync.dma_start(taps["a2a_out0"][:], a2a_out[0][:])
        nc.sync.dma_start(taps["a2a_out1"][:], a2a_out[1][:])

    # ---- Phase C: projection (+bias) on the gathered 2x256-token slice
    lt_pool = ctx.enter_context(tc.tile_pool(name="lt", bufs=3))
    ob_pool = ctx.enter_context(tc.tile_pool(name="ob", bufs=2))
    with tc.tile_pool(name="psO", bufs=4, space="PSUM") as psO_pool:
        for tt in range(4):
            b_, st = tt // 2, tt % 2
            po = psO_pool.tile([128, D], F32, tag="po", name=f"po_{tt}")
            for dc in range(8):
                lt = lt_pool.tile([128, 128], BF16, tag="lt", name=f"lt_{dc}_{tt}")
                nc.sync.dma_start(
                    lt[:], a2a_out[b_][ts(dc, 128), ts(st, 128)]
                )
                for nh in range(2):
                    nc.tensor.matmul(
                        po[:, ts(nh, 512)],
                        lhsT=lt[:],
                        rhs=wpj_sb[dc][:, ts(nh, 512)],
                        start=(dc == 0),
                        stop=False,
                    )
            for nh in range(2):
                nc.tensor.matmul(
                    po[:, ts(nh, 512)],
                    lhsT=ones1[0:1, :],
                    rhs=bias_sb[0:1, ts(nh, 512)],
                    start=False,
                    stop=True,
                )
            ob = ob_pool.tile([128, D], F32, tag="ob", name=f"ob_{tt}")
            nc.scalar.copy(ob[:], po[:])
            nc.sync.dma_start(out[ts(tt, 128), :], ob[:])


def build_program(with_taps=False):
    nc = bacc.Bacc("TRN2", target_bir_lowering=False, debug=False, num_devices=N_CORES)
    xT = nc.dram_tensor("xT", [D, T], BF16, kind="ExternalInput").ap()
    w_qkv = nc.dram_tensor("w_qkv", [D, 384], BF16, kind="ExternalInput").ap()
    w_pj = nc.dram_tensor("w_pj", [D, D], BF16, kind="ExternalInput").ap()
    b_pj = nc.dram_tensor("b_pj", [1, D], BF16, kind="ExternalInput").ap()
    ident = nc.dram_tensor("ident", [128, 128], BF16, kind="ExternalInput").ap()
    # rows 0-255 = batch-0 stripe, rows 256-511 = batch-1 stripe
    out = nc.dram_tensor("out", [2 * STRIPE, D], F32, kind="ExternalOutput").ap()

    taps = None
    if with_taps:
        taps = {
            "qk0": nc.dram_tensor("tap_qk0", [128, T], BF16, kind="ExternalOutput").ap(),
            "qk1": nc.dram_tensor("tap_qk1", [128, T], BF16, kind="ExternalOutput").ap(),
            "v": nc.dram_tensor(
                "tap_v", [128, 2 * B * N_KC * 64], BF16, kind="ExternalOutput"
            ).ap(),
            "a2a_out0": nc.dram_tensor(
                "tap_a2a_out0", [N_CORES * 128, STRIPE], BF16, kind="ExternalOutput"
            ).ap(),
            "a2a_out1": nc.dram_tensor(
                "tap_a2a_out1", [N_CORES * 128, STRIPE], BF16, kind="ExternalOutput"
            ).ap(),
        }
        for b_ in range(B):
            taps[f"outT{b_}"] = nc.dram_tensor(
                f"tap_outT{b_}", [128, S], BF16, kind="ExternalOutput"
            ).ap()

    from contextlib import ExitStack

    with tile.TileContext(nc) as tc:
        with ExitStack() as ctx:
            build_attention_body(ctx, tc, xT, w_qkv, w_pj, b_pj, ident, out, taps=taps)
    nc.compile()
    return nc


_NC_CACHE = None


def _get_program():
    global _NC_CACHE
    if _NC_CACHE is None:
        _NC_CACHE = build_program()
    return _NC_CACHE


def make_in_maps(x, w_qkv, w_proj, b_proj):
    import ml_dtypes

    bf16 = ml_dtypes.bfloat16
    x = np.asarray(x, dtype=np.float32)
    w_qkv = np.asarray(w_qkv, dtype=np.float32)
    w_proj = np.asarray(w_proj, dtype=np.float32)
    b_proj = np.asarray(b_proj, dtype=np.float32)

    xT = np.ascontiguousarray(x.reshape(T, D).T).astype(bf16)  # [1024, 4096]
    w_pjT = np.ascontiguousarray(w_proj.T).astype(bf16)  # [1024, 1024]
    b_row = np.ascontiguousarray(b_proj.reshape(1, D)).astype(bf16)
    ident = np.eye(128, dtype=bf16)
    wq = w_qkv[0:D]
    wk = w_qkv[D : 2 * D]
    wv = w_qkv[2 * D : 3 * D]

    in_maps = []
    for c in range(N_CORES):
        h0, h1 = 2 * c, 2 * c + 1
        w_qkv_c = np.concatenate(
            [
                wq[HD * h0 : HD * h0 + HD] * SCALE,
                wq[HD * h1 : HD * h1 + HD] * SCALE,
                wk[HD * h0 : HD * h0 + HD],
                wk[HD * h1 : HD * h1 + HD],
                wv[HD * h0 : HD * h0 + HD],
                wv[HD * h1 : HD * h1 + HD],
            ],
            axis=0,
        )  # [384, 1024]
        in_maps.append(
            {
                "xT": xT,
                "w_qkv": np.ascontiguousarray(w_qkv_c.T).astype(bf16),
                "w_pj": w_pjT,
                "b_pj": b_row,
                "ident": ident,
            }
        )
    return in_maps


def assemble_output(results):
    out = np.empty((B, S, D), np.float32)
    for c in range(N_CORES):
        out[0, STRIPE * c : STRIPE * (c + 1), :] = results[c]["out"][0:STRIPE]
        out[1, STRIPE * c : STRIPE * (c + 1), :] = results[c]["out"][STRIPE:]
    return out


def kernel(x, w_qkv, w_proj, b_proj):
    nc = _get_program()
    in_maps = make_in_maps(x, w_qkv, w_proj, b_proj)
    res = run_bass_kernel_spmd(nc, in_maps, list(range(N_CORES)))
    return assemble_output(res.results)


# revision 23
# speedup vs baseline: 1.0542x; 1.0542x over previous
"""Multi-head attention forward on 8 Trainium2 NeuronCores (Bass/Tile).

Problem: x[2,2048,1024] -> qkv (w_qkv [3072,1024]) -> 16-head softmax attention
-> proj (w_proj [1024,1024] + b_proj) -> out[2,2048,1024], fp32.

Sharding: head-parallel. Core c computes heads (2c, 2c+1) for BOTH batches:
its slice of the QKV projection, its heads' attention, then two 8-core
AllToAlls (one per batch; the batch-0 one overlaps batch-1 attention)
redistribute the attention output from head-split to token-split, after which
each core does the full-depth output projection (+bias) for its 2x256-token
striped slice. Host assembles the 16 stripes.

Layout: everything flows transposed (d-major / "T" form):
  - host feeds xT [1024, 4096] and per-core w_qkvT [1024, 384] (softmax scale
    folded into the q columns), w_projT [1024, 1024], all bf16 (fp32 matmuls
    stream at half rate on the PE).
  - QKV phase emits QT/KT/VT [128 (2 heads x 64d), 4096 tokens] e-major; VT is
    then PE-transposed per 128-token chunk into V [token-major, 2 x (64 + ones
    column)] for the PV matmul (col 64 of each 65-block -> softmax denom).
  - scores are computed transposed (scoresT[k, q] = sum_d KT[d,k] QT[d,q]),
    two heads row-packed in the PE array (K=64 each, rows 0-63 / 64-127).
    Softmax runs WITHOUT max subtraction (logits ~ N(0,1); exp is safe in
    fp32), and the PV matmul's ones-column yields the denominators free.
"""

import numpy as np

import concourse.bass as bass
import concourse.tile as tile
from concourse import bacc, mybir
from concourse.bass_utils import run_bass_kernel_spmd

F32 = mybir.dt.float32
BF16 = mybir.dt.bfloat16
ts = bass.ts

B = 2
S = 2048
D = 1024
H = 16
HD = 64
SCALE = HD ** -0.5
T = B * S  # 4096 tokens
N_CORES = 8

TT = 512  # phase-A token tile (matmul N)
N_TT = T // TT  # 8
KC = 128  # attention key-chunk (contraction tile for PV)
N_KC = S // KC  # 16 per batch
QT = 1024  # attention query tile (exp instruction width)
N_QT = S // QT  # 2 per batch
STRIPE = 256  # per-core tokens per batch (striped output slices)


def build_attention_body(ctx, tc, xT, w_qkv, w_pj, b_pj, ident, out, taps=None):
    nc = tc.nc
    EXP = mybir.ActivationFunctionType.Exp

    singles = ctx.enter_context(tc.tile_pool(name="singles", bufs=1))
    dram = ctx.enter_context(tc.tile_pool(name="dram", bufs=1, space="DRAM"))

    # persistent SBUF
    # qk_sb[0] = QT-pair [128 (h0 d0-63 | h1 d0-63), 4096], qk_sb[1] = KT-pair
    qk_sb = [singles.tile([128, T], BF16, name=f"qk_sb{i}") for i in range(2)]
    # V: per 128-token chunk kcg and head hh, [128 k, 64] at cols 64*(2*kcg+hh)
    v_sb = singles.tile([128, 2 * B * N_KC * 64], BF16, name="v_sb")
    # attention output (normalized), d-major: head A rows 0-63, head B 64-127
    outT = [singles.tile([128, S], BF16, name=f"outT_{b_}") for b_ in range(B)]
    wqkv_sb = [singles.tile([128, 384], BF16, name=f"wqkv_sb{dc}") for dc in range(8)]
    wpj_sb = [singles.tile([128, D], BF16, name=f"wpj_sb{dc}") for dc in range(8)]
    bias_sb = singles.tile([1, D], BF16, name="bias_sb")
    ones1 = singles.tile([1, 128], BF16, name="ones1")
    ones_col = singles.tile([128, 1], BF16, name="ones_col")
    id_sb = singles.tile([128, 128], BF16, name="id_sb")

    # DRAM bounce buffers for the two AllToAlls (batch 0, batch 1)
    a2a_in = [
        dram.tile([N_CORES * 128, STRIPE], BF16, name=f"a2a_in{b_}") for b_ in range(B)
    ]
    a2a_out = [
        dram.tile([N_CORES * 128, STRIPE], BF16, name=f"a2a_out{b_}") for b_ in range(B)
    ]
    rdram = ctx.enter_context(tc.tile_pool(name="rdram", bufs=4, space="DRAM"))

    # ---- load weights / constants
    for dc in range(8):
        nc.sync.dma_start(wqkv_sb[dc][:], w_qkv[ts(dc, 128), :])
        nc.sync.dma_start(wpj_sb[dc][:], w_pj[ts(dc, 128), :])
    nc.sync.dma_start(bias_sb[:], b_pj[:])
    nc.sync.dma_start(id_sb[:], ident[:])
    nc.vector.memset(ones1[:], 1.0)
    nc.vector.memset(ones_col[:], 1.0)

    # ---- Phase A: QKV projections, all e-major; V transposed on the PE
    # preload all of xT up front: 8 x 1MB DMAs (small tiled loads pay ~1us
    # per-DMA overhead and made phase A DMA-bound)
    x_sb = [singles.tile([128, T], BF16, name=f"x_sb{dc}") for dc in range(8)]
    for dc in range(8):
        nc.sync.dma_start(x_sb[dc][:], xT[ts(dc, 128), :])
    vt_pool = ctx.enter_context(tc.tile_pool(name="vt", bufs=2))
    with tc.tile_pool(name="psA", bufs=3, space="PSUM") as psA_pool:
        with tc.tile_pool(name="psT", bufs=2, space="PSUM") as psT_pool:
            for tt in range(N_TT):
                # out[e, t] = sum_d w_qkvT[d, e] * xT[d, t], e-tiles: Q | K | V
                vt_t = vt_pool.tile([128, TT], BF16, tag="vt", name=f"vt_{tt}")
                for et in range(3):
                    ps = psA_pool.tile([128, TT], F32, tag="qk", name=f"ps_{tt}_{et}")
                    for dc in range(8):
                        nc.tensor.matmul(
                            ps[:],
                            lhsT=wqkv_sb[dc][:, ts(et, 128)],
                            rhs=x_sb[dc][:, ts(tt, TT)],
                            start=(dc == 0),
                            stop=(dc == 7),
                        )
                    if et < 2:
                        nc.scalar.copy(qk_sb[et][:, ts(tt, TT)], ps[:])
                    else:
                        nc.scalar.copy(vt_t[:], ps[:])
                # transpose VT 128-token chunks into v_sb
                for st in range(TT // 128):
                    kcg = tt * (TT // 128) + st  # global 128-token chunk 0..31
                    tp = psT_pool.tile([128, 128], BF16, tag="tp", name=f"tp_{kcg}")
                    nc.tensor.transpose(tp[:], vt_t[:, ts(st, 128)], id_sb[:])
                    nc.vector.tensor_copy(
                        v_sb[:, 128 * kcg : 128 * kcg + 128], tp[:]
                    )

    if taps is not None:
        nc.sync.dma_start(taps["qk0"][:], qk_sb[0][:])
        nc.sync.dma_start(taps["qk1"][:], qk_sb[1][:])
        nc.sync.dma_start(taps["v"][:], v_sb[:])

    # ---- Phase B: attention per (batch, query-tile); batch-b A2A fires as
    # soon as batch b's divisions are done (b=0's overlaps b=1's attention)
    probs = ctx.enter_context(tc.tile_pool(name="probs", bufs=2))
    misc = ctx.enter_context(tc.tile_pool(name="misc", bufs=2))
    with tc.tile_pool(name="psS", bufs=2, space="PSUM") as psS_pool:
        with tc.tile_pool(name="psP", bufs=2, space="PSUM") as psP_pool:
            with tc.tile_pool(name="psD", bufs=2, space="PSUM") as psD_pool:
                for b_ in range(B):
                    for qt in range(N_QT):
                        q0 = S * b_ + QT * qt
                        # col-packed accumulators: head A rows 0-63, head B
                        # rows 64-127 of the SAME bank. Only head A's kc==0
                        # matmul uses start=True (it clears the whole bank's
                        # has_written bits); head B always start=False and
                        # relies on overwrite-where-unset for its first write.
                        pvs = [
                            psP_pool.tile(
                                [128, 512], F32, tag="pv", name=f"pv_{b_}_{qt}_{qs}"
                            )
                            for qs in range(QT // 512)
                        ]
                        dens = [
                            psD_pool.tile(
                                [128, 512], F32, tag="den", name=f"den_{b_}_{qt}_{qs}"
                            )
                            for qs in range(QT // 512)
                        ]
                        # zero head B's regions: its first matmul never uses
                        # start=True (that would clear head A's has_written
                        # bits bank-wide); overwrite-where-unset on HW and
                        # plain accumulate in CoreSim both land on the right
                        # values when the region starts at zero
                        for qs in range(QT // 512):
                            nc.vector.memset(pvs[qs][:], 0.0)
                            # 1.0 (not 0.0) so the reciprocal over the unused
                            # rows 1-63 stays finite for the simulator; head
                            # B's first den matmul overwrites row 64 on HW
                            # (has_written unset) and accumulates onto 1.0 in
                            # CoreSim (negligible: denominators are ~2e3)
                            nc.vector.memset(dens[qs][:], 1.0)

                        def emit_pv(kc, prs, b_=b_, pvs=pvs, dens=dens):
                            first = kc == 0
                            last = kc == N_KC - 1
                            for qs in range(QT // 512):
                                for hh in range(2):
                                    blk = 64 * (2 * (N_KC * b_ + kc) + hh)
                                    nc.tensor.matmul(
                                        pvs[qs][64 * hh : 64 * hh + 64, :],
                                        lhsT=v_sb[:, blk : blk + 64],
                                        rhs=prs[hh][:, ts(qs, 512)],
                                        start=(first and hh == 0),
                                        stop=(last and hh == 1),
                                        skip_group_check=True,
                                    )
                            for qs in range(QT // 512):
                                for hh in range(2):
                                    nc.tensor.matmul(
                                        dens[qs][64 * hh : 64 * hh + 1, :],
                                        lhsT=ones_col[:],
                                        rhs=prs[hh][:, ts(qs, 512)],
                                        start=(first and hh == 0),
                                        stop=(last and hh == 1),
                                        skip_group_check=True,
                                    )

                        prev = None
                        for kc in range(N_KC):
                            k0 = S * b_ + KC * kc
                            scs = [
                                psS_pool.tile(
                                    [128, QT], F32, tag=f"sc{hh}", bufs=1,
                                    name=f"sc_{b_}_{qt}_{kc}_{hh}",
                                )
                                for hh in range(2)
                            ]
                            # interleave heads so row-packed pairs co-issue
                            for qs in range(QT // 512):
                                for hh in range(2):
                                    p0 = 64 * hh
                                    nc.tensor.matmul(
                                        scs[hh][:, ts(qs, 512)],
                                        lhsT=qk_sb[1][p0 : p0 + 64, k0 : k0 + KC],
                                        rhs=qk_sb[0][
                                            p0 : p0 + 64,
                                            q0 + 512 * qs : q0 + 512 * qs + 512,
                                        ],
                                        start=True,
                                        stop=True,
                                    )
                            prs = []
                            for hh in range(2):
                                pr = probs.tile(
                                    [128, QT], BF16, tag=f"pr{hh}", bufs=3,
                                    name=f"pr_{b_}_{qt}_{kc}_{hh}",
                                )
                                nc.scalar.activation(pr[:], scs[hh][:], EXP)
                                prs.append(pr)
                            if prev is not None:
                                emit_pv(kc - 1, prev)
                            prev = prs
                        emit_pv(N_KC - 1, prev)

                        # normalize: rows 0-63 (head A) / 64-127 (head B) by
                        # the denominators in dens rows 0 / 64
                        for qs in range(QT // 512):
                            den_t = dens[qs]
                            pv_t = pvs[qs]
                            # the approx reciprocal must start at partition
                            # 0 (custom-DVE ops mis-execute at base_partition
                            # != 0 on HW); rows 1-63 hold the memset 1.0
                            rc = misc.tile(
                                [128, 512], F32, tag="rc", name=f"rc_{b_}_{qt}_{qs}"
                            )
                            nc.vector.reciprocal_approx_fast(
                                rc[0:65, :], den_t[0:65, :]
                            )
                            bc = misc.tile(
                                [128, 512], F32, tag="bc", name=f"bc_{b_}_{qt}_{qs}"
                            )
                            for hh in range(2):
                                rd = rdram.tile(
                                    [1, 512], F32, tag="rd",
                                    name=f"rd_{b_}_{qt}_{qs}_{hh}",
                                )
                                nc.sync.dma_start(rd[:], rc[64 * hh : 64 * hh + 1, :])
                                nc.sync.dma_start(
                                    bc[64 * hh : 64 * hh + 64, :],
                                    rd[:].to_broadcast((64, 512)),
                                )
                            qq = QT * qt + 512 * qs
                            for hh in range(2):
                                sl = slice(64 * hh, 64 * hh + 64)
                                nc.vector.tensor_mul(
                                    outT[b_][sl, qq : qq + 512],
                                    pv_t[sl, :],
                                    bc[sl, :],
                                )

                    # batch b_ fully normalized -> ship its AllToAll now
                    for j in range(N_CORES):
                        nc.sync.dma_start(
                            a2a_in[b_][128 * j : 128 * j + 128, :],
                            outT[b_][:, ts(j, STRIPE)],
                        )
                    nc.gpsimd.collective_compute(
                        "AllToAll",
                        mybir.AluOpType.bypass,
                        replica_groups=[list(range(N_CORES))],
                        ins=[a2a_in[b_].opt()],
                        outs=[a2a_out[b_].opt()],
                    )

    if taps is not None:
        for b_ in range(B):
            nc.sync.dma_start(taps[f"outT{b_}"][:], outT[b_][:])
        nc.sync.dma_start(taps["a2a_out0"][:], a2a_out[0][:])
        nc.sync.dma_start(taps["a2a_out1"][:], a2a_out[1][:])

    # ---- Phase C: projection (+bias) on the gathered 2x256-token slice
    lt_pool = ctx.enter_context(tc.tile_pool(name="lt", bufs=3))
    ob_pool = ctx.enter_context(tc.tile_pool(name="ob", bufs=2))
    with tc.tile_pool(name="psO", bufs=4, space="PSUM") as psO_pool:
        for tt in range(4):
            b_, st = tt // 2, tt % 2
            po = psO_pool.tile([128, D], F32, tag="po", name=f"po_{tt}")
            for dc in range(8):
                lt = lt_pool.tile([128, 128], BF16, tag="lt", name=f"lt_{dc}_{tt}")
                nc.sync.dma_start(
                    lt[:], a2a_out[b_][ts(dc, 128), ts(st, 128)]
                )
                for nh in range(2):
                    nc.tensor.matmul(
                        po[:, ts(nh, 512)],
                        lhsT=lt[:],
                        rhs=wpj_sb[dc][:, ts(nh, 512)],
                        start=(dc == 0),
                        stop=False,
                    )
            for nh in range(2):
                nc.tensor.matmul(
                    po[:, ts(nh, 512)],
                    lhsT=ones1[0:1, :],
                    rhs=bias_sb[0:1, ts(nh, 512)],
                    start=False,
                    stop=True,
                )
            ob = ob_pool.tile([128, D], F32, tag="ob", name=f"ob_{tt}")
            nc.scalar.copy(ob[:], po[:])
            nc.sync.dma_start(out[ts(tt, 128), :], ob[:])


def build_program(with_taps=False):
    nc = bacc.Bacc("TRN2", target_bir_lowering=False, debug=False, num_devices=N_CORES)
    xT = nc.dram_tensor("xT", [D, T], BF16, kind="ExternalInput").ap()
    w_qkv = nc.dram_tensor("w_qkv", [D, 384], BF16, kind="ExternalInput").ap()
    w_pj = nc.dram_tensor("w_pj", [D, D], BF16, kind="ExternalInput").ap()
    b_pj = nc.dram_tensor("b_pj", [1, D], BF16, kind="ExternalInput").ap()
    ident = nc.dram_tensor("ident", [128, 128], BF16, kind="ExternalInput").ap()
    # rows 0-255 = batch-0 stripe, rows 256-511 = batch-1 stripe
    out = nc.dram_tensor("out", [2 * STRIPE, D], F32, kind="ExternalOutput").ap()

    taps = None
    if with_taps:
        taps = {
            "qk0": nc.dram_tensor("tap_qk0", [128, T], BF16, kind="ExternalOutput").ap(),
            "qk1": nc.dram_tensor("tap_qk1", [128, T], BF16, kind="ExternalOutput").ap(),
            "v": nc.dram_tensor(
                "tap_v", [128, 2 * B * N_KC * 64], BF16, kind="ExternalOutput"
            ).ap(),
            "a2a_out0": nc.dram_tensor(
                "tap_a2a_out0", [N_CORES * 128, STRIPE], BF16, kind="ExternalOutput"
            ).ap(),
            "a2a_out1": nc.dram_tensor(
                "tap_a2a_out1", [N_CORES * 128, STRIPE], BF16, kind="ExternalOutput"
            ).ap(),
        }
        for b_ in range(B):
            taps[f"outT{b_}"] = nc.dram_tensor(
                f"tap_outT{b_}", [128, S], BF16, kind="ExternalOutput"
            ).ap()

    from contextlib import ExitStack

    with tile.TileContext(nc) as tc:
        with ExitStack() as ctx:
            build_attention_body(ctx, tc, xT, w_qkv, w_pj, b_pj, ident, out, taps=taps)
    nc.compile()
    return nc


_NC_CACHE = None


def _get_program():
    global _NC_CACHE
    if _NC_CACHE is None:
        _NC_CACHE = build_program()
    return _NC_CACHE


def make_in_maps(x, w_qkv, w_proj, b_proj):
    import ml_dtypes

    bf16 = ml_dtypes.bfloat16
    x = np.asarray(x, dtype=np.float32)
    w_qkv = np.asarray(w_qkv, dtype=np.float32)
    w_proj = np.asarray(w_proj, dtype=np.float32)
    b_proj = np.asarray(b_proj, dtype=np.float32)

    xT = np.ascontiguousarray(x.reshape(T, D).T).astype(bf16)  # [1024, 4096]
    w_pjT = np.ascontiguousarray(w_proj.T).astype(bf16)  # [1024, 1024]
    b_row = np.ascontiguousarray(b_proj.reshape(1, D)).astype(bf16)
    ident = np.eye(128, dtype=bf16)
    wq = w_qkv[0:D]
    wk = w_qkv[D : 2 * D]
    wv = w_qkv[2 * D : 3 * D]

    in_maps = []
    for c in range(N_CORES):
        h0, h1 = 2 * c, 2 * c + 1
        w_qkv_c = np.concatenate(
            [
                wq[HD * h0 : HD * h0 + HD] * SCALE,
                wq[HD * h1 : HD * h1 + HD] * SCALE,
                wk[HD * h0 : HD * h0 + HD],
                wk[HD * h1 : HD * h1 + HD],
                wv[HD * h0 : HD * h0 + HD],
                wv[HD * h1 : HD * h1 + HD],
            ],
            axis=0,
        )  # [384, 1024]
        in_maps.append(
            {
                "xT": xT,
                "w_qkv": np.ascontiguousarray(w_qkv_c.T).astype(bf16),
                "w_pj": w_pjT,
                "b_pj": b_row,
                "ident": ident,
            }
        )
    return in_maps


def assemble_output(results):
    out = np.empty((B, S, D), np.float32)
    for c in range(N_CORES):
        out[0, STRIPE * c : STRIPE * (c + 1), :] = results[c]["out"][0:STRIPE]
        out[1, STRIPE * c : STRIPE * (c + 1), :] = results[c]["out"][STRIPE:]
    return out


def kernel(x, w_qkv, w_proj, b_proj):
    nc = _get_program()
    in_maps = make_in_maps(x, w_qkv, w_proj, b_proj)
    res = run_bass_kernel_spmd(nc, in_maps, list(range(N_CORES)))
    return assemble_output(res.results)
